# revision 8
# baseline (speedup 1.0000x reference)
"""Trainium2 Bass kernel for nn_EnhancedDLinear (8-core SPMD, full I/O).

v2: single mega-packed bf16 input tensor (big DMA rows, need-ordered
chunks across 4 issue queues), f32 biases bit-packed into bf16 columns
(read back via AP bitcast), exp+accum_out and gpsimd partition
all-reduce for the softmax denominator, single [48,192]-row output DMA.

Mathematical reductions (vs the jax reference, verified numerically):
1. LayerNorm(1) output is the constant ln_b, so the detail branch is a
   weight-only constant row folded on the host.
2. The replicate-pad moving average (k=25) is folded into the first
   trend/seasonal MLP layers.
3. The channel-mean feeding the fusion MLP folds into its weights; the
   constant detail contribution folds into its bias.
4. Biases ride matmuls via constant-one contraction rows, activation
   bias operands come from bit-packed f32 columns of the mega tensor.
5. The fusion softmax normalizer folds into the final hidden Relu's
   per-partition scale; its denominator comes from exp-accum + gpsimd
   partition all-reduce (no extra PE matmul/reduce chain).

Sharding: one batch per core (N = B*C, contiguous blocks of C=96), zero
collectives, weights replicated.
"""

import numpy as np
import ml_dtypes

import concourse.bacc as bacc
import concourse.tile as tile
from concourse import mybir
from concourse import bass_isa
from concourse.bass_utils import run_bass_kernel_spmd

B, S, C, P = 8, 336, 96, 96
HID = 168
MAIN_K = 25
N_CORES = 8
KC = 112          # contraction chunk (336 = 3*112)

# mega tensor column layout (bf16 cols)
XB0 = 0            # xb [112, 288]
B1B = 288          # b1 f32-bits [112, 6] (2 cols per u), 294:296 pad
WA0 = 296          # wa u-major [112, 3*336] -> 296:1304
SUM0 = 1304        # L2 sum col pairs [113, 6] (2 per u)
FN1T = 1310        # fn1t-aug [97, 32]
FN1S = 1342        # fn1s-aug [97, 32]
WIDE0 = 1374       # L2 wide [113, 3*192]
FN2 = 1950         # fn2aug [33, 288]
FP1W = 2238        # fp1wT [96, 48]
DPB = 2286         # dpb [96, 96]
FP2 = 2382         # fp2aug [49, 96]
FP1B = 2478        # fp1b f32-bits [48, 2]
MCOLS = 2480

_CACHE = {}


def _mavg_matrix(s, k):
    p = (k - 1) // 2
    m = np.zeros((s, s), np.float64)
    for j in range(s):
        for d in range(-p, p + 1):
            i = min(max(j + d, 0), s - 1)
            m[i, j] += 1.0 / k
    return m.astype(np.float32)


def _bf(a):
    return np.ascontiguousarray(a, np.float32).astype(ml_dtypes.bfloat16)


def _f32bits(a):
    # f32 array [..., n] -> bf16-bit view [..., 2n] (little-endian halves)
    a = np.ascontiguousarray(a, np.float32)
    return a.view(np.uint16).view(ml_dtypes.bfloat16)


def _build_module():
    f32 = mybir.dt.float32
    bf16 = mybir.dt.bfloat16
    nc = bacc.Bacc("TRN2", target_bir_lowering=False, debug=False,
                   num_devices=N_CORES)

    # one dram tensor per DMA chunk: contiguous sources (strided column
    # slices of a wide tensor transfer 3-4x slower and stall the DGE ring)
    T1 = nc.dram_tensor("T1", [113, WA0], bf16, kind="ExternalInput")
    T2 = nc.dram_tensor("T2", [113, S], bf16, kind="ExternalInput")
    T3 = nc.dram_tensor("T3", [113, S], bf16, kind="ExternalInput")
    T4 = nc.dram_tensor("T4", [113, S], bf16, kind="ExternalInput")
    T5 = nc.dram_tensor("T5", [113, WIDE0 - SUM0], bf16, kind="ExternalInput")
    T6 = nc.dram_tensor("T6", [113, 384], bf16, kind="ExternalInput")
    T7 = nc.dram_tensor("T7", [113, 192], bf16, kind="ExternalInput")
    T8 = nc.dram_tensor("T8", [33, 288], bf16, kind="ExternalInput")
    T9 = nc.dram_tensor("T9", [96, MCOLS - FP1W], bf16, kind="ExternalInput")
    y = nc.dram_tensor("y", [48, 192], f32, kind="ExternalOutput")

    AF = mybir.ActivationFunctionType

    with tile.TileContext(nc) as tc:
        with (
            tc.tile_pool(name="wp", bufs=1) as wp,
            tc.tile_pool(name="hp", bufs=1) as hp,
            tc.tile_pool(name="pp", bufs=7, space="PSUM") as pp,
        ):
            Ms = wp.tile([113, MCOLS], bf16, tag="Ms")

            # need-ordered chunk DMAs; HWDGE queues are sync + scalar only.
            # sync:   xb+b1 | u1 | sums+fn1 | wide-u0,u1 | fn2
            # scalar: u0 | u2 | wide-u2 | fp1w+dpb+fp2+fp1b
            nc.sync.dma_start(out=Ms[:, 0:WA0], in_=T1[:, :])
            nc.scalar.dma_start(out=Ms[:, WA0:WA0 + S], in_=T2[:, :])
            nc.sync.dma_start(out=Ms[:, WA0 + S:WA0 + 2 * S], in_=T3[:, :])
            nc.scalar.dma_start(out=Ms[:, WA0 + 2 * S:WA0 + 3 * S],
                                in_=T4[:, :])
            nc.sync.dma_start(out=Ms[:, SUM0:WIDE0], in_=T5[:, :])
            nc.sync.dma_start(out=Ms[:, WIDE0:WIDE0 + 384], in_=T6[:, :])
            nc.scalar.dma_start(out=Ms[:, WIDE0 + 384:FN2], in_=T7[:, :])
            nc.sync.dma_start(out=Ms[0:33, FN2:FP1W], in_=T8[:, :])
            nc.scalar.dma_start(out=Ms[0:96, FP1W:MCOLS], in_=T9[:, :])

            # constant-one rows (gpsimd: off the DMA-issue queues)
            h1c0 = hp.tile([KC, 96], bf16, tag="h1c0")
            h1c1 = hp.tile([KC, 96], bf16, tag="h1c1")
            h1c2 = hp.tile([KC + 1, 96], bf16, tag="h1c2")
            ts2 = hp.tile([97, 2], bf16, tag="ts2")
            z1s = hp.tile([33, 1], bf16, tag="z1s")
            hs = hp.tile([49, 96], bf16, tag="hs")
            nc.gpsimd.memset(h1c2[:, :], 1.0)
            nc.gpsimd.memset(ts2[:, :], 1.0)
            nc.gpsimd.memset(z1s[:, :], 1.0)
            nc.gpsimd.memset(hs[:, :], 1.0)

            b1u = [Ms[:, B1B + 2 * u:B1B + 2 * (u + 1)].bitcast(f32)
                   for u in range(3)]
            wa_u = lambda u, j: Ms[:, WA0 + S * u + KC * j:
                                   WA0 + S * u + KC * (j + 1)]
            xb_j = lambda j: Ms[0:KC, C * j:C * (j + 1)]

            # ---- layer 1: u-major so each psum closes early ----
            psu = [pp.tile([KC, 96], f32, tag="ps", name=f"psu{u}")
                   for u in range(3)]
            for u in range(3):
                for j in range(3):
                    nc.tensor.matmul(psu[u], wa_u(u, j)[0:KC, :], xb_j(j),
                                     start=(j == 0), stop=(j == 2))
            nc.scalar.activation(h1c0, psu[0], AF.Relu, bias=b1u[0][0:KC, :])
            nc.vector.tensor_scalar(h1c1, psu[1], b1u[1][0:KC, :], 0.0,
                                    mybir.AluOpType.add, mybir.AluOpType.max)
            nc.scalar.activation(h1c2[0:KC, :], psu[2], AF.Relu,
                                 bias=b1u[2][0:KC, :])

            # ---- L2 sum-cols first (feeds the serial softmax chain) ----
            ps_sums = pp.tile([96, 2], f32, tag="ps", name="ps_sums")
            nc.tensor.matmul(ps_sums, h1c0, Ms[0:KC, SUM0:SUM0 + 2],
                             start=True, stop=False)
            nc.tensor.matmul(ps_sums, h1c1, Ms[0:KC, SUM0 + 2:SUM0 + 4],
                             start=False, stop=False)
            nc.tensor.matmul(ps_sums, h1c2, Ms[0:KC + 1, SUM0 + 4:SUM0 + 6],
                             start=False, stop=True)
            nc.scalar.activation(ts2[0:96, :], ps_sums, AF.Copy)

            # ---- z1 = relu(fn1 @ ts2 + b1f) (bias rides aug row 96) ----
            ps_z1 = pp.tile([32, 1], f32, tag="ps", name="ps_z1")
            nc.tensor.matmul(ps_z1, Ms[0:97, FN1T:FN1T + 32], ts2[:, 0:1],
                             start=True, stop=False)
            nc.tensor.matmul(ps_z1, Ms[0:97, FN1S:FN1S + 32], ts2[:, 1:2],
                             start=False, stop=True)

            # ---- wide L2 [tp | sp] ----
            ps_l2 = pp.tile([96, 192], f32, tag="ps", name="ps_l2")
            nc.tensor.matmul(ps_l2, h1c0, Ms[0:KC, WIDE0:WIDE0 + 192],
                             start=True, stop=False)
            nc.tensor.matmul(ps_l2, h1c1, Ms[0:KC, WIDE0 + 192:WIDE0 + 384],
                             start=False, stop=False)
            nc.tensor.matmul(ps_l2, h1c2,
                             Ms[0:KC + 1, WIDE0 + 384:WIDE0 + 576],
                             start=False, stop=True)

            nc.vector.tensor_scalar(z1s[0:32, :], ps_z1, 0.0, None,
                                    mybir.AluOpType.max)

            # ---- zc = fn2 @ z1 (cols per k), then exp + accum ----
            zc = pp.tile([96, 3], f32, tag="ps", name="zc")
            for k in range(3):
                nc.tensor.matmul(zc[:, k:k + 1],
                                 Ms[0:33, FN2 + 96 * k:FN2 + 96 * (k + 1)],
                                 z1s, skip_group_check=True)
            at_s = hp.tile([96, 96], bf16, tag="at_s")
            nc.vector.tensor_copy(at_s, ps_l2[:, 0:96])
            asl_s = hp.tile([96, 96], bf16, tag="asl_s")
            nc.scalar.activation(asl_s, ps_l2[:, 96:192], AF.Copy)

            ec = hp.tile([96, 3], f32, tag="ec")
            ecsum = hp.tile([96, 1], f32, tag="ecsum")
            nc.scalar.activation(ec, zc, AF.Exp, accum_out=ecsum)

            # denominator -> per-partition recip for the Relu scale
            ecsb = hp.tile([96, 1], bf16, tag="ecsb")
            nc.gpsimd.tensor_copy(ecsb, ecsum)
            ones48 = hp.tile([96, 48], bf16, tag="ones48")
            nc.gpsimd.memset(ones48[:, :], 1.0)
            den48 = pp.tile([48, 1], f32, tag="ps", name="den48")
            nc.tensor.matmul(den48, ones48, ecsb, start=True, stop=True)
            recip48 = hp.tile([48, 1], f32, tag="recip48")
            nc.vector.reciprocal(recip48, den48)

            # e-weights fold into small [96,48] stationary muls
            fp1wT = Ms[0:96, FP1W:FP1W + 48]
            wt = hp.tile([96, 48], bf16, tag="wt")
            nc.vector.tensor_scalar_mul(wt, fp1wT, ec[:, 0:1])
            ws = hp.tile([96, 48], bf16, tag="ws")
            nc.scalar.activation(ws, fp1wT, AF.Copy, scale=ec[:, 1:2])
            wd = hp.tile([96, 48], bf16, tag="wd")
            nc.gpsimd.tensor_scalar_mul(wd, fp1wT, ec[:, 2:3])

            # ps_h = fp1w @ (e0*tp + e1*sp + e2*dp), unnormalized
            ps_h = pp.tile([48, 96], f32, tag="ps", name="ps_h")
            nc.tensor.matmul(ps_h, wt, at_s, start=True, stop=False)
            nc.tensor.matmul(ps_h, wd, Ms[0:96, DPB:DPB + 96],
                             start=False, stop=False)
            nc.tensor.matmul(ps_h, ws, asl_s, start=False, stop=True)

            nc.scalar.activation(hs[0:48, :], ps_h, AF.Relu,
                                 bias=Ms[0:48, FP1B:FP1B + 2].bitcast(f32),
                                 scale=recip48)

            # output split by stationary halves so both PSUM tiles sit on
            # partitions 0:48 and the copies stay partition-aligned
            ps_oA = pp.tile([48, 96], f32, tag="ps", name="ps_oA")
            ps_oB = pp.tile([48, 96], f32, tag="ps", name="ps_oB")
            nc.tensor.matmul(ps_oA, hs[:, 0:48], Ms[0:49, FP2:FP2 + 96],
                             start=True, stop=True)
            nc.tensor.matmul(ps_oB, hs[:, 48:96], Ms[0:49, FP2:FP2 + 96],
                             start=True, stop=True)
            out48 = hp.tile([48, 192], f32, tag="out")
            nc.vector.tensor_copy(out48[:, 0:96], ps_oA)
            nc.scalar.activation(out48[:, 96:192], ps_oB, AF.Copy)
            nc.sync.dma_start(out=y[:, :], in_=out48)

    nc.compile()
    return nc


def _prep_shared():
    f = np.float32
    mm = _mavg_matrix(S, MAIN_K)
    Msh = np.zeros((113, MCOLS), f)

    # constants derived from weights (filled in _prep_weights)
    return Msh


def _prep_weights(i):
    f = np.float32
    mm = _mavg_matrix(S, MAIN_K)
    w1 = np.empty((S, 2 * HID), f)
    w1[:, :HID] = mm @ i['lt1w'].T.astype(f)
    w1[:, HID:] = (np.eye(S, dtype=f) - mm) @ i['ls1w'].T.astype(f)

    Msh = np.zeros((113, MCOLS), f)
    Mbf = np.zeros((113, MCOLS), ml_dtypes.bfloat16)

    # wa u-major blocks
    for u in range(3):
        for j in range(3):
            Msh[0:KC, WA0 + S * u + KC * j:WA0 + S * u + KC * (j + 1)] = \
                w1[KC * j:KC * (j + 1), KC * u:KC * (u + 1)]

    # constant detail_pred row (LayerNorm(1) output == ln_b exactly)
    xf = np.full((S,), f(i['ln_b'][0]), f)
    dp_row = (np.maximum(xf @ i['op1w'].T + i['op1b'], 0)
              @ i['op2w'].T + i['op2b']).astype(f)
    dpm = dp_row.mean(dtype=np.float32)
    b1f = (i['fn1b'] + dpm * i['fn1w'][:, 2 * C:].sum(1)).astype(f)

    lt2wt = np.ascontiguousarray(i['lt2w'].T, f)
    ls2wt = np.ascontiguousarray(i['ls2w'].T, f)
    # w2full [337, 194] = [tp 0:96 | sp 96:192 | tps 192 | sps 193]
    w2full = np.zeros((S + 1, 194), f)
    w2full[0:HID, 0:96] = lt2wt
    w2full[0:HID, 192] = lt2wt.sum(1)
    w2full[HID:S, 96:192] = ls2wt
    w2full[HID:S, 193] = ls2wt.sum(1)
    w2full[S, 0:96] = i['lt2b']
    w2full[S, 192] = i['lt2b'].sum(dtype=np.float64)
    w2full[S, 96:192] = i['ls2b']
    w2full[S, 193] = i['ls2b'].sum(dtype=np.float64)
    for u in range(3):
        Msh[0:KC, WIDE0 + 192 * u:WIDE0 + 192 * (u + 1)] = \
            w2full[KC * u:KC * (u + 1), 0:192]
        Msh[0:KC, SUM0 + 2 * u:SUM0 + 2 * (u + 1)] = \
            w2full[KC * u:KC * (u + 1), 192:194]
    Msh[KC, WIDE0 + 384:WIDE0 + 576] = w2full[S, 0:192]
    Msh[KC, SUM0 + 4:SUM0 + 6] = w2full[S, 192:194]

    # fn1 aug (bias b1f rides row 96 of fn1t; ts2 row 96 == 1)
    Msh[0:96, FN1T:FN1T + 32] = i['fn1w'][:, 0:C].T / C
    Msh[96, FN1T:FN1T + 32] = b1f
    Msh[0:96, FN1S:FN1S + 32] = i['fn1w'][:, C:2 * C].T / C

    fn2T = np.ascontiguousarray(i['fn2w'].T, f)
    Msh[0:32, FN2:FN2 + 288] = fn2T
    Msh[32, FN2:FN2 + 288] = i['fn2b']

    Msh[0:96, FP1W:FP1W + 48] = i['fp1w'].T
    Msh[0:96, DPB:DPB + 96] = np.broadcast_to(dp_row[None, :], (96, 96))
    Msh[0:48, FP2:FP2 + 96] = i['fp2w'].T
    Msh[48, FP2:FP2 + 96] = i['fp2b']

    Mbf[:, :] = Msh.astype(ml_dtypes.bfloat16)

    # f32-bit-packed columns (overwrite the bf16 rounding)
    b1 = np.concatenate([i['lt1b'], i['ls1b']]).astype(f)
    for u in range(3):
        Mbf[0:KC, B1B + 2 * u:B1B + 2 * (u + 1)] = \
            _f32bits(b1[KC * u:KC * (u + 1)][:, None])
    Mbf[0:48, FP1B:FP1B + 2] = _f32bits(i['fp1b'].astype(f)[:, None])

    return Mbf


def _chunks(Mbf):
    cc = np.ascontiguousarray
    return dict(
        T2=cc(Mbf[:, WA0:WA0 + S]),
        T3=cc(Mbf[:, WA0 + S:WA0 + 2 * S]),
        T4=cc(Mbf[:, WA0 + 2 * S:WA0 + 3 * S]),
        T5=cc(Mbf[:, SUM0:WIDE0]),
        T6=cc(Mbf[:, WIDE0:WIDE0 + 384]),
        T7=cc(Mbf[:, WIDE0 + 384:FN2]),
        T8=cc(Mbf[0:33, FN2:FP1W]),
        T9=cc(Mbf[0:96, FP1W:MCOLS]),
    )


def make_in_maps(inputs):
    Mbf = _prep_weights(inputs)
    shared = _chunks(Mbf)
    x = np.asarray(inputs['x'], np.float32)
    in_maps = []
    for b in range(N_CORES):
        T1 = np.ascontiguousarray(Mbf[:, 0:WA0])
        for j in range(3):
            T1[0:KC, C * j:C * (j + 1)] = _bf(x[b, KC * j:KC * (j + 1), :])
        in_maps.append(dict(shared, T1=T1))
    return in_maps


def kernel(**inputs):
    if "nc" not in _CACHE:
        _CACHE["nc"] = _build_module()
    res = run_bass_kernel_spmd(_CACHE["nc"], make_in_maps(inputs),
                               core_ids=list(range(N_CORES)))
    out = np.empty((N_CORES, P, P), np.float32)
    for b in range(N_CORES):
        y2 = res.results[b]["y"]
        out[b, 0:48, :] = y2[:, 0:96]
        out[b, 48:96, :] = y2[:, 96:192]
    return out


# revision 12
# speedup vs baseline: 1.7364x; 1.7364x over previous
"""Trainium2 Bass kernel for nn_EnhancedDLinear (8-core SPMD, full I/O).

v2: single mega-packed bf16 input tensor (big DMA rows, need-ordered
chunks across 4 issue queues), f32 biases bit-packed into bf16 columns
(read back via AP bitcast), exp+accum_out and gpsimd partition
all-reduce for the softmax denominator, single [48,192]-row output DMA.

Mathematical reductions (vs the jax reference, verified numerically):
1. LayerNorm(1) output is the constant ln_b, so the detail branch is a
   weight-only constant row folded on the host.
2. The replicate-pad moving average (k=25) is folded into the first
   trend/seasonal MLP layers.
3. The channel-mean feeding the fusion MLP folds into its weights; the
   constant detail contribution folds into its bias.
4. Biases ride matmuls via constant-one contraction rows, activation
   bias operands come from bit-packed f32 columns of the mega tensor.
5. The fusion softmax normalizer folds into the final hidden Relu's
   per-partition scale; its denominator comes from exp-accum + gpsimd
   partition all-reduce (no extra PE matmul/reduce chain).

Sharding: one batch per core (N = B*C, contiguous blocks of C=96), zero
collectives, weights replicated.
"""

import numpy as np
import ml_dtypes

import concourse.bacc as bacc
import concourse.tile as tile
from concourse import mybir
from concourse import bass_isa
from concourse.bass_utils import run_bass_kernel_spmd

B, S, C, P = 8, 336, 96, 96
HID = 168
MAIN_K = 25
N_CORES = 8
KC = 112          # contraction chunk (336 = 3*112)

# mega tensor column layout (bf16 cols)
XB0 = 0            # xb [112, 288]
B1B = 288          # b1 f32-bits [112, 6] (2 cols per u), 294:296 pad
WA0 = 296          # wa u-major [112, 3*336] -> 296:1304
SUM0 = 1304        # L2 sum col pairs [113, 6] (2 per u)
FN1T = 1310        # fn1t-aug [97, 32]
FN1S = 1342        # fn1s-aug [97, 32]
WIDE0 = 1374       # L2 wide [113, 3*192]
FN2 = 1950         # fn2aug [33, 288]
FP1W = 2238        # fp1wT [96, 48]
DPB = 2286         # dpb [96, 96]
FP2 = 2382         # fp2aug [49, 96]
FP1B = 2478        # fp1b f32-bits [48, 2]
MCOLS = 2480

_CACHE = {}


def _mavg_matrix(s, k):
    p = (k - 1) // 2
    m = np.zeros((s, s), np.float64)
    for j in range(s):
        for d in range(-p, p + 1):
            i = min(max(j + d, 0), s - 1)
            m[i, j] += 1.0 / k
    return m.astype(np.float32)


def _bf(a):
    return np.ascontiguousarray(a, np.float32).astype(ml_dtypes.bfloat16)


def _f32bits(a):
    # f32 array [..., n] -> bf16-bit view [..., 2n] (little-endian halves)
    a = np.ascontiguousarray(a, np.float32)
    return a.view(np.uint16).view(ml_dtypes.bfloat16)


def _build_module():
    f32 = mybir.dt.float32
    bf16 = mybir.dt.bfloat16
    nc = bacc.Bacc("TRN2", target_bir_lowering=False, debug=False,
                   num_devices=N_CORES)

    # one dram tensor per DMA chunk: contiguous sources (strided column
    # slices of a wide tensor transfer 3-4x slower and stall the DGE ring).
    # Row counts MUST be multiples of 16 or the transfer serializes onto a
    # single DMA engine instead of striping across all 16.
    T1 = nc.dram_tensor("T1", [112, WA0], bf16, kind="ExternalInput")
    T2 = nc.dram_tensor("T2", [112, S], bf16, kind="ExternalInput")
    T3 = nc.dram_tensor("T3", [112, S], bf16, kind="ExternalInput")
    T4 = nc.dram_tensor("T4", [112, S], bf16, kind="ExternalInput")
    T5 = nc.dram_tensor("T5", [128, WIDE0 - SUM0], bf16, kind="ExternalInput")
    T6 = nc.dram_tensor("T6", [112, 384], bf16, kind="ExternalInput")
    T7 = nc.dram_tensor("T7", [128, 192], bf16, kind="ExternalInput")
    T8 = nc.dram_tensor("T8", [48, 288], bf16, kind="ExternalInput")
    T9 = nc.dram_tensor("T9", [96, MCOLS - FP1W], bf16, kind="ExternalInput")
    y = nc.dram_tensor("y", [48, 192], f32, kind="ExternalOutput")

    AF = mybir.ActivationFunctionType

    with tile.TileContext(nc) as tc:
        with (
            tc.tile_pool(name="wp", bufs=1) as wp,
            tc.tile_pool(name="hp", bufs=1) as hp,
            tc.tile_pool(name="pp", bufs=7, space="PSUM") as pp,
        ):
            Ms = wp.tile([128, MCOLS], bf16, tag="Ms")

            # need-ordered chunk DMAs; HWDGE queues are sync + scalar only.
            # sync:   xb+b1 | u1 | sums+fn1 | wide-u0,u1 | fn2
            # scalar: u0 | u2 | wide-u2 | fp1w+dpb+fp2+fp1b
            nc.sync.dma_start(out=Ms[0:112, 0:WA0], in_=T1[:, :])
            nc.scalar.dma_start(out=Ms[0:112, WA0:WA0 + S], in_=T2[:, :])
            nc.sync.dma_start(out=Ms[0:112, WA0 + S:WA0 + 2 * S], in_=T3[:, :])
            nc.scalar.dma_start(out=Ms[0:112, WA0 + 2 * S:WA0 + 3 * S],
                                in_=T4[:, :])
            nc.sync.dma_start(out=Ms[0:128, SUM0:WIDE0], in_=T5[:, :])
            nc.sync.dma_start(out=Ms[0:112, WIDE0:WIDE0 + 384], in_=T6[:, :])
            nc.scalar.dma_start(out=Ms[0:128, WIDE0 + 384:FN2], in_=T7[:, :])
            nc.sync.dma_start(out=Ms[0:48, FN2:FP1W], in_=T8[:, :])
            nc.scalar.dma_start(out=Ms[0:96, FP1W:MCOLS], in_=T9[:, :])

            # constant-one rows (gpsimd: off the DMA-issue queues)
            h1c0 = hp.tile([KC, 96], bf16, tag="h1c0")
            h1c1 = hp.tile([KC, 96], bf16, tag="h1c1")
            h1c2 = hp.tile([KC + 1, 96], bf16, tag="h1c2")
            ts2 = hp.tile([97, 2], bf16, tag="ts2")
            z1s = hp.tile([33, 1], bf16, tag="z1s")
            hs = hp.tile([49, 96], bf16, tag="hs")
            nc.gpsimd.memset(h1c2[:, :], 1.0)
            nc.gpsimd.memset(ts2[:, :], 1.0)
            nc.gpsimd.memset(z1s[:, :], 1.0)
            nc.gpsimd.memset(hs[:, :], 1.0)

            b1u = [Ms[:, B1B + 2 * u:B1B + 2 * (u + 1)].bitcast(f32)
                   for u in range(3)]
            wa_u = lambda u, j: Ms[:, WA0 + S * u + KC * j:
                                   WA0 + S * u + KC * (j + 1)]
            xb_j = lambda j: Ms[0:KC, C * j:C * (j + 1)]

            # ---- layer 1: u-major so each psum closes early ----
            psu = [pp.tile([KC, 96], f32, tag="ps", name=f"psu{u}")
                   for u in range(3)]
            for u in range(3):
                for j in range(3):
                    nc.tensor.matmul(psu[u], wa_u(u, j)[0:KC, :], xb_j(j),
                                     start=(j == 0), stop=(j == 2))
            nc.scalar.activation(h1c0, psu[0], AF.Relu, bias=b1u[0][0:KC, :])
            nc.vector.tensor_scalar(h1c1, psu[1], b1u[1][0:KC, :], 0.0,
                                    mybir.AluOpType.add, mybir.AluOpType.max)
            nc.scalar.activation(h1c2[0:KC, :], psu[2], AF.Relu,
                                 bias=b1u[2][0:KC, :])

            # ---- L2 sum-cols first (feeds the serial softmax chain) ----
            ps_sums = pp.tile([96, 2], f32, tag="ps", name="ps_sums")
            nc.tensor.matmul(ps_sums, h1c0, Ms[0:KC, SUM0:SUM0 + 2],
                             start=True, stop=False)
            nc.tensor.matmul(ps_sums, h1c1, Ms[0:KC, SUM0 + 2:SUM0 + 4],
                             start=False, stop=False)
            nc.tensor.matmul(ps_sums, h1c2, Ms[0:KC + 1, SUM0 + 4:SUM0 + 6],
                             start=False, stop=True)
            nc.scalar.activation(ts2[0:96, :], ps_sums, AF.Copy)

            # ---- z1 = relu(fn1 @ ts2 + b1f) (bias rides aug row 96) ----
            ps_z1 = pp.tile([32, 1], f32, tag="ps", name="ps_z1")
            nc.tensor.matmul(ps_z1, Ms[0:97, FN1T:FN1T + 32], ts2[:, 0:1],
                             start=True, stop=False)
            nc.tensor.matmul(ps_z1, Ms[0:97, FN1S:FN1S + 32], ts2[:, 1:2],
                             start=False, stop=True)

            # ---- wide L2 [tp | sp] ----
            ps_l2 = pp.tile([96, 192], f32, tag="ps", name="ps_l2")
            nc.tensor.matmul(ps_l2, h1c0, Ms[0:KC, WIDE0:WIDE0 + 192],
                             start=True, stop=False)
            nc.tensor.matmul(ps_l2, h1c1, Ms[0:KC, WIDE0 + 192:WIDE0 + 384],
                             start=False, stop=False)
            nc.tensor.matmul(ps_l2, h1c2,
                             Ms[0:KC + 1, WIDE0 + 384:WIDE0 + 576],
                             start=False, stop=True)

            nc.vector.tensor_scalar(z1s[0:32, :], ps_z1, 0.0, None,
                                    mybir.AluOpType.max)

            # ---- zc = fn2 @ z1 (cols per k), then exp + accum ----
            zc = pp.tile([96, 3], f32, tag="ps", name="zc")
            for k in range(3):
                nc.tensor.matmul(zc[:, k:k + 1],
                                 Ms[0:33, FN2 + 96 * k:FN2 + 96 * (k + 1)],
                                 z1s, skip_group_check=True)
            at_s = hp.tile([96, 96], bf16, tag="at_s")
            nc.vector.tensor_copy(at_s, ps_l2[:, 0:96])
            asl_s = hp.tile([96, 96], bf16, tag="asl_s")
            nc.scalar.activation(asl_s, ps_l2[:, 96:192], AF.Copy)

            ec = hp.tile([96, 3], f32, tag="ec")
            ecsum = hp.tile([96, 1], f32, tag="ecsum")
            nc.scalar.activation(ec, zc, AF.Exp, accum_out=ecsum)

            # denominator -> per-partition recip for the Relu scale
            ecsb = hp.tile([96, 1], bf16, tag="ecsb")
            nc.gpsimd.tensor_copy(ecsb, ecsum)
            ones48 = hp.tile([96, 48], bf16, tag="ones48")
            nc.gpsimd.memset(ones48[:, :], 1.0)
            den48 = pp.tile([48, 1], f32, tag="ps", name="den48")
            nc.tensor.matmul(den48, ones48, ecsb, start=True, stop=True)
            recip48 = hp.tile([48, 1], f32, tag="recip48")
            nc.vector.reciprocal(recip48, den48)

            # e-weights fold into small [96,48] stationary muls
            fp1wT = Ms[0:96, FP1W:FP1W + 48]
            wt = hp.tile([96, 48], bf16, tag="wt")
            nc.vector.tensor_scalar_mul(wt, fp1wT, ec[:, 0:1])
            ws = hp.tile([96, 48], bf16, tag="ws")
            nc.scalar.activation(ws, fp1wT, AF.Copy, scale=ec[:, 1:2])
            wd = hp.tile([96, 48], bf16, tag="wd")
            nc.gpsimd.tensor_scalar_mul(wd, fp1wT, ec[:, 2:3])

            # ps_h = fp1w @ (e0*tp + e1*sp + e2*dp), unnormalized
            ps_h = pp.tile([48, 96], f32, tag="ps", name="ps_h")
            nc.tensor.matmul(ps_h, wt, at_s, start=True, stop=False)
            nc.tensor.matmul(ps_h, wd, Ms[0:96, DPB:DPB + 96],
                             start=False, stop=False)
            nc.tensor.matmul(ps_h, ws, asl_s, start=False, stop=True)

            nc.scalar.activation(hs[0:48, :], ps_h, AF.Relu,
                                 bias=Ms[0:48, FP1B:FP1B + 2].bitcast(f32),
                                 scale=recip48)

            # output split by stationary halves so both PSUM tiles sit on
            # partitions 0:48 and the copies stay partition-aligned
            ps_oA = pp.tile([48, 96], f32, tag="ps", name="ps_oA")
            ps_oB = pp.tile([48, 96], f32, tag="ps", name="ps_oB")
            nc.tensor.matmul(ps_oA, hs[:, 0:48], Ms[0:49, FP2:FP2 + 96],
                             start=True, stop=True)
            nc.tensor.matmul(ps_oB, hs[:, 48:96], Ms[0:49, FP2:FP2 + 96],
                             start=True, stop=True)
            out48 = hp.tile([48, 192], f32, tag="out")
            nc.vector.tensor_copy(out48[:, 0:96], ps_oA)
            nc.scalar.activation(out48[:, 96:192], ps_oB, AF.Copy)
            nc.sync.dma_start(out=y[:, :], in_=out48)

    nc.compile()
    return nc


def _prep_shared():
    f = np.float32
    mm = _mavg_matrix(S, MAIN_K)
    Msh = np.zeros((113, MCOLS), f)

    # constants derived from weights (filled in _prep_weights)
    return Msh


def _prep_weights(i):
    f = np.float32
    mm = _mavg_matrix(S, MAIN_K)
    w1 = np.empty((S, 2 * HID), f)
    w1[:, :HID] = mm @ i['lt1w'].T.astype(f)
    w1[:, HID:] = (np.eye(S, dtype=f) - mm) @ i['ls1w'].T.astype(f)

    Msh = np.zeros((128, MCOLS), f)
    Mbf = np.zeros((128, MCOLS), ml_dtypes.bfloat16)

    # wa u-major blocks
    for u in range(3):
        for j in range(3):
            Msh[0:KC, WA0 + S * u + KC * j:WA0 + S * u + KC * (j + 1)] = \
                w1[KC * j:KC * (j + 1), KC * u:KC * (u + 1)]

    # constant detail_pred row (LayerNorm(1) output == ln_b exactly)
    xf = np.full((S,), f(i['ln_b'][0]), f)
    dp_row = (np.maximum(xf @ i['op1w'].T + i['op1b'], 0)
              @ i['op2w'].T + i['op2b']).astype(f)
    dpm = dp_row.mean(dtype=np.float32)
    b1f = (i['fn1b'] + dpm * i['fn1w'][:, 2 * C:].sum(1)).astype(f)

    lt2wt = np.ascontiguousarray(i['lt2w'].T, f)
    ls2wt = np.ascontiguousarray(i['ls2w'].T, f)
    # w2full [337, 194] = [tp 0:96 | sp 96:192 | tps 192 | sps 193]
    w2full = np.zeros((S + 1, 194), f)
    w2full[0:HID, 0:96] = lt2wt
    w2full[0:HID, 192] = lt2wt.sum(1)
    w2full[HID:S, 96:192] = ls2wt
    w2full[HID:S, 193] = ls2wt.sum(1)
    w2full[S, 0:96] = i['lt2b']
    w2full[S, 192] = i['lt2b'].sum(dtype=np.float64)
    w2full[S, 96:192] = i['ls2b']
    w2full[S, 193] = i['ls2b'].sum(dtype=np.float64)
    for u in range(3):
        Msh[0:KC, WIDE0 + 192 * u:WIDE0 + 192 * (u + 1)] = \
            w2full[KC * u:KC * (u + 1), 0:192]
        Msh[0:KC, SUM0 + 2 * u:SUM0 + 2 * (u + 1)] = \
            w2full[KC * u:KC * (u + 1), 192:194]
    Msh[KC, WIDE0 + 384:WIDE0 + 576] = w2full[S, 0:192]
    Msh[KC, SUM0 + 4:SUM0 + 6] = w2full[S, 192:194]

    # fn1 aug (bias b1f rides row 96 of fn1t; ts2 row 96 == 1)
    Msh[0:96, FN1T:FN1T + 32] = i['fn1w'][:, 0:C].T / C
    Msh[96, FN1T:FN1T + 32] = b1f
    Msh[0:96, FN1S:FN1S + 32] = i['fn1w'][:, C:2 * C].T / C

    fn2T = np.ascontiguousarray(i['fn2w'].T, f)
    Msh[0:32, FN2:FN2 + 288] = fn2T
    Msh[32, FN2:FN2 + 288] = i['fn2b']

    Msh[0:96, FP1W:FP1W + 48] = i['fp1w'].T
    Msh[0:96, DPB:DPB + 96] = np.broadcast_to(dp_row[None, :], (96, 96))
    Msh[0:48, FP2:FP2 + 96] = i['fp2w'].T
    Msh[48, FP2:FP2 + 96] = i['fp2b']

    Mbf[:, :] = Msh.astype(ml_dtypes.bfloat16)

    # f32-bit-packed columns (overwrite the bf16 rounding)
    b1 = np.concatenate([i['lt1b'], i['ls1b']]).astype(f)
    for u in range(3):
        Mbf[0:KC, B1B + 2 * u:B1B + 2 * (u + 1)] = \
            _f32bits(b1[KC * u:KC * (u + 1)][:, None])
    Mbf[0:48, FP1B:FP1B + 2] = _f32bits(i['fp1b'].astype(f)[:, None])

    return Mbf


def _chunks(Mbf):
    cc = np.ascontiguousarray
    return dict(
        T2=cc(Mbf[0:112, WA0:WA0 + S]),
        T3=cc(Mbf[0:112, WA0 + S:WA0 + 2 * S]),
        T4=cc(Mbf[0:112, WA0 + 2 * S:WA0 + 3 * S]),
        T5=cc(Mbf[0:128, SUM0:WIDE0]),
        T6=cc(Mbf[0:112, WIDE0:WIDE0 + 384]),
        T7=cc(Mbf[0:128, WIDE0 + 384:FN2]),
        T8=cc(Mbf[0:48, FN2:FP1W]),
        T9=cc(Mbf[0:96, FP1W:MCOLS]),
    )


def make_in_maps(inputs):
    Mbf = _prep_weights(inputs)
    shared = _chunks(Mbf)
    x = np.asarray(inputs['x'], np.float32)
    in_maps = []
    for b in range(N_CORES):
        T1 = np.ascontiguousarray(Mbf[0:112, 0:WA0])
        for j in range(3):
            T1[0:KC, C * j:C * (j + 1)] = _bf(x[b, KC * j:KC * (j + 1), :])
        in_maps.append(dict(shared, T1=T1))
    return in_maps


def kernel(**inputs):
    if "nc" not in _CACHE:
        _CACHE["nc"] = _build_module()
    res = run_bass_kernel_spmd(_CACHE["nc"], make_in_maps(inputs),
                               core_ids=list(range(N_CORES)))
    out = np.empty((N_CORES, P, P), np.float32)
    for b in range(N_CORES):
        y2 = res.results[b]["y"]
        out[b, 0:48, :] = y2[:, 0:96]
        out[b, 48:96, :] = y2[:, 96:192]
    return out


# revision 16
# speedup vs baseline: 1.8130x; 1.0441x over previous
"""Trainium2 Bass kernel for nn_EnhancedDLinear (8-core SPMD, full I/O).

v2: single mega-packed bf16 input tensor (big DMA rows, need-ordered
chunks across 4 issue queues), f32 biases bit-packed into bf16 columns
(read back via AP bitcast), exp+accum_out and gpsimd partition
all-reduce for the softmax denominator, single [48,192]-row output DMA.

Mathematical reductions (vs the jax reference, verified numerically):
1. LayerNorm(1) output is the constant ln_b, so the detail branch is a
   weight-only constant row folded on the host.
2. The replicate-pad moving average (k=25) is folded into the first
   trend/seasonal MLP layers.
3. The channel-mean feeding the fusion MLP folds into its weights; the
   constant detail contribution folds into its bias.
4. Biases ride matmuls via constant-one contraction rows, activation
   bias operands come from bit-packed f32 columns of the mega tensor.
5. The fusion softmax normalizer folds into the final hidden Relu's
   per-partition scale; its denominator comes from exp-accum + gpsimd
   partition all-reduce (no extra PE matmul/reduce chain).

Sharding: one batch per core (N = B*C, contiguous blocks of C=96), zero
collectives, weights replicated.
"""

import numpy as np
import ml_dtypes

import concourse.bacc as bacc
import concourse.tile as tile
from concourse import mybir
from concourse import bass_isa
from concourse.bass_utils import run_bass_kernel_spmd

B, S, C, P = 8, 336, 96, 96
HID = 168
MAIN_K = 25
N_CORES = 8
KC = 112          # contraction chunk (336 = 3*112)

# mega tensor column layout (bf16 cols)
XB0 = 0            # xb [112, 288]
B1B = 288          # b1 f32-bits [112, 6] (2 cols per u), 294:296 pad
WA0 = 296          # wa u-major [112, 3*336] -> 296:1304
SUM0 = 1304        # L2 sum col pairs [113, 6] (2 per u)
FN1T = 1310        # fn1t-aug [97, 32]
FN1S = 1342        # fn1s-aug [97, 32]
WIDE0 = 1374       # L2 wide [113, 3*192]
FN2 = 1950         # fn2aug [33, 288]
FP1W = 2238        # fp1wT [96, 48]
DPB = 2286         # dpb [96, 96]
FP2 = 2382         # fp2aug [49, 96]
FP1B = 2478        # fp1b f32-bits [48, 2]
MCOLS = 2480

_CACHE = {}


def _mavg_matrix(s, k):
    p = (k - 1) // 2
    m = np.zeros((s, s), np.float64)
    for j in range(s):
        for d in range(-p, p + 1):
            i = min(max(j + d, 0), s - 1)
            m[i, j] += 1.0 / k
    return m.astype(np.float32)


def _bf(a):
    return np.ascontiguousarray(a, np.float32).astype(ml_dtypes.bfloat16)


def _f32bits(a):
    # f32 array [..., n] -> bf16-bit view [..., 2n] (little-endian halves)
    a = np.ascontiguousarray(a, np.float32)
    return a.view(np.uint16).view(ml_dtypes.bfloat16)


def _build_module():
    f32 = mybir.dt.float32
    bf16 = mybir.dt.bfloat16
    nc = bacc.Bacc("TRN2", target_bir_lowering=False, debug=False,
                   num_devices=N_CORES)

    # one dram tensor per DMA chunk: contiguous sources (strided column
    # slices of a wide tensor transfer 3-4x slower and stall the DGE ring).
    # Row counts MUST be multiples of 16 or the transfer serializes onto a
    # single DMA engine instead of striping across all 16.
    T1 = nc.dram_tensor("T1", [112, WA0], bf16, kind="ExternalInput")
    T2 = nc.dram_tensor("T2", [112, S], bf16, kind="ExternalInput")
    T3 = nc.dram_tensor("T3", [112, S], bf16, kind="ExternalInput")
    T4 = nc.dram_tensor("T4", [112, S], bf16, kind="ExternalInput")
    T5 = nc.dram_tensor("T5", [128, WIDE0 - SUM0], bf16, kind="ExternalInput")
    T6a = nc.dram_tensor("T6a", [112, 192], bf16, kind="ExternalInput")
    T6b = nc.dram_tensor("T6b", [128, 384], bf16, kind="ExternalInput")
    T8 = nc.dram_tensor("T8", [48, 288], bf16, kind="ExternalInput")
    T9 = nc.dram_tensor("T9", [96, MCOLS - FP1W], bf16, kind="ExternalInput")
    y = nc.dram_tensor("y", [48, 192], f32, kind="ExternalOutput")

    AF = mybir.ActivationFunctionType

    with tile.TileContext(nc) as tc:
        with (
            tc.tile_pool(name="wp", bufs=1) as wp,
            tc.tile_pool(name="hp", bufs=1) as hp,
            tc.tile_pool(name="pp", bufs=7, space="PSUM") as pp,
        ):
            Ms = wp.tile([128, MCOLS], bf16, tag="Ms")

            # constant-one rows first on gpsimd, then its SWDGE DMAs
            h1c0 = hp.tile([KC, 96], bf16, tag="h1c0")
            h1c1 = hp.tile([KC, 96], bf16, tag="h1c1")
            h1c2 = hp.tile([KC + 1, 96], bf16, tag="h1c2")
            ts2 = hp.tile([97, 2], bf16, tag="ts2")
            z1s = hp.tile([33, 1], bf16, tag="z1s")
            hs = hp.tile([49, 96], bf16, tag="hs")
            nc.gpsimd.memset(h1c2[:, :], 1.0)
            nc.gpsimd.memset(ts2[:, :], 1.0)
            nc.gpsimd.memset(z1s[:, :], 1.0)
            nc.gpsimd.memset(hs[:, :], 1.0)

            # need-ordered chunk DMAs across three queues:
            # sync:   xb+b1 | u1 | wide-u1,u2
            # scalar: u0 | u2 | wide-u0
            # gpsimd (SWDGE): sums+fn1 | fn2 | fp1w+dpb+fp2+fp1b
            nc.sync.dma_start(out=Ms[0:112, 0:WA0], in_=T1[:, :])
            nc.scalar.dma_start(out=Ms[0:112, WA0:WA0 + S], in_=T2[:, :])
            nc.sync.dma_start(out=Ms[0:112, WA0 + S:WA0 + 2 * S], in_=T3[:, :])
            nc.scalar.dma_start(out=Ms[0:112, WA0 + 2 * S:WA0 + 3 * S],
                                in_=T4[:, :])
            nc.gpsimd.dma_start(out=Ms[0:128, SUM0:WIDE0], in_=T5[:, :])
            nc.scalar.dma_start(out=Ms[0:112, WIDE0:WIDE0 + 192],
                                in_=T6a[:, :])
            nc.sync.dma_start(out=Ms[0:128, WIDE0 + 192:FN2], in_=T6b[:, :])
            nc.gpsimd.dma_start(out=Ms[0:48, FN2:FP1W], in_=T8[:, :])
            nc.gpsimd.dma_start(out=Ms[0:96, FP1W:MCOLS], in_=T9[:, :])

            b1u = [Ms[:, B1B + 2 * u:B1B + 2 * (u + 1)].bitcast(f32)
                   for u in range(3)]
            wa_u = lambda u, j: Ms[:, WA0 + S * u + KC * j:
                                   WA0 + S * u + KC * (j + 1)]
            xb_j = lambda j: Ms[0:KC, C * j:C * (j + 1)]

            # ---- layer 1: u-major so each psum closes early ----
            psu = [pp.tile([KC, 96], f32, tag="ps", name=f"psu{u}")
                   for u in range(3)]
            for u in range(3):
                for j in range(3):
                    nc.tensor.matmul(psu[u], wa_u(u, j)[0:KC, :], xb_j(j),
                                     start=(j == 0), stop=(j == 2))
            nc.scalar.activation(h1c0, psu[0], AF.Relu, bias=b1u[0][0:KC, :])
            nc.vector.tensor_scalar(h1c1, psu[1], b1u[1][0:KC, :], 0.0,
                                    mybir.AluOpType.add, mybir.AluOpType.max)
            nc.vector.tensor_scalar(h1c2[0:KC, :], psu[2], b1u[2][0:KC, :],
                                    0.0, mybir.AluOpType.add,
                                    mybir.AluOpType.max)

            # ---- L2 sum-cols first (feeds the serial softmax chain) ----
            ps_sums = pp.tile([96, 2], f32, tag="ps", name="ps_sums")
            nc.tensor.matmul(ps_sums, h1c0, Ms[0:KC, SUM0:SUM0 + 2],
                             start=True, stop=False)
            nc.tensor.matmul(ps_sums, h1c1, Ms[0:KC, SUM0 + 2:SUM0 + 4],
                             start=False, stop=False)
            nc.tensor.matmul(ps_sums, h1c2, Ms[0:KC + 1, SUM0 + 4:SUM0 + 6],
                             start=False, stop=True)
            nc.scalar.activation(ts2[0:96, :], ps_sums, AF.Copy)

            # ---- wide L2 [tp | sp], z1 mms slotted between chunks ----
            ps_l2 = pp.tile([96, 192], f32, tag="ps", name="ps_l2")
            ps_z1 = pp.tile([32, 1], f32, tag="ps", name="ps_z1")
            nc.tensor.matmul(ps_l2, h1c0, Ms[0:KC, WIDE0:WIDE0 + 192],
                             start=True, stop=False)
            nc.tensor.matmul(ps_z1, Ms[0:97, FN1T:FN1T + 32], ts2[:, 0:1],
                             start=True, stop=False)
            nc.tensor.matmul(ps_z1, Ms[0:97, FN1S:FN1S + 32], ts2[:, 1:2],
                             start=False, stop=True)
            nc.tensor.matmul(ps_l2, h1c1, Ms[0:KC, WIDE0 + 192:WIDE0 + 384],
                             start=False, stop=False)
            nc.tensor.matmul(ps_l2, h1c2,
                             Ms[0:KC + 1, WIDE0 + 384:WIDE0 + 576],
                             start=False, stop=True)

            nc.vector.tensor_scalar(z1s[0:32, :], ps_z1, 0.0, None,
                                    mybir.AluOpType.max)

            # ---- zc = fn2 @ z1 (cols per k), then exp + accum ----
            zc = pp.tile([96, 3], f32, tag="ps", name="zc")
            for k in range(3):
                nc.tensor.matmul(zc[:, k:k + 1],
                                 Ms[0:33, FN2 + 96 * k:FN2 + 96 * (k + 1)],
                                 z1s, skip_group_check=True)
            at_s = hp.tile([96, 96], bf16, tag="at_s")
            nc.vector.tensor_copy(at_s, ps_l2[:, 0:96])
            asl_s = hp.tile([96, 96], bf16, tag="asl_s")
            nc.vector.tensor_copy(asl_s, ps_l2[:, 96:192])

            ec = hp.tile([96, 3], f32, tag="ec")
            ecsum = hp.tile([96, 1], f32, tag="ecsum")
            nc.scalar.activation(ec, zc, AF.Exp, accum_out=ecsum)

            # denominator -> per-partition recip for the Relu scale
            d96 = hp.tile([96, 1], f32, tag="d96")
            nc.gpsimd.partition_all_reduce(d96, ecsum, 96,
                                           bass_isa.ReduceOp.add)

            # e-weights fold into small [96,48] stationary muls
            fp1wT = Ms[0:96, FP1W:FP1W + 48]
            wt = hp.tile([96, 48], bf16, tag="wt")
            nc.vector.tensor_scalar_mul(wt, fp1wT, ec[:, 0:1])
            wd = hp.tile([96, 48], bf16, tag="wd")
            nc.vector.tensor_scalar_mul(wd, fp1wT, ec[:, 2:3])
            ws = hp.tile([96, 48], bf16, tag="ws")
            nc.scalar.activation(ws, fp1wT, AF.Copy, scale=ec[:, 1:2])
            recip48 = hp.tile([48, 1], f32, tag="recip48")
            nc.vector.reciprocal(recip48, d96[0:48, :])

            # ps_h = fp1w @ (e0*tp + e1*sp + e2*dp), unnormalized
            ps_h = pp.tile([48, 96], f32, tag="ps", name="ps_h")
            nc.tensor.matmul(ps_h, wt, at_s, start=True, stop=False)
            nc.tensor.matmul(ps_h, wd, Ms[0:96, DPB:DPB + 96],
                             start=False, stop=False)
            nc.tensor.matmul(ps_h, ws, asl_s, start=False, stop=True)

            nc.scalar.activation(hs[0:48, :], ps_h, AF.Relu,
                                 bias=Ms[0:48, FP1B:FP1B + 2].bitcast(f32),
                                 scale=recip48)

            # output split by stationary halves so both PSUM tiles sit on
            # partitions 0:48 and the copies stay partition-aligned
            ps_oA = pp.tile([48, 96], f32, tag="ps", name="ps_oA")
            ps_oB = pp.tile([48, 96], f32, tag="ps", name="ps_oB")
            nc.tensor.matmul(ps_oA, hs[:, 0:48], Ms[0:49, FP2:FP2 + 96],
                             start=True, stop=True)
            nc.tensor.matmul(ps_oB, hs[:, 48:96], Ms[0:49, FP2:FP2 + 96],
                             start=True, stop=True)
            out48 = hp.tile([48, 192], f32, tag="out")
            nc.vector.tensor_copy(out48[:, 0:96], ps_oA)
            nc.scalar.activation(out48[:, 96:192], ps_oB, AF.Copy)
            nc.sync.dma_start(out=y[:, :], in_=out48)

    nc.compile()
    return nc


def _prep_shared():
    f = np.float32
    mm = _mavg_matrix(S, MAIN_K)
    Msh = np.zeros((113, MCOLS), f)

    # constants derived from weights (filled in _prep_weights)
    return Msh


def _prep_weights(i):
    f = np.float32
    mm = _mavg_matrix(S, MAIN_K)
    w1 = np.empty((S, 2 * HID), f)
    w1[:, :HID] = mm @ i['lt1w'].T.astype(f)
    w1[:, HID:] = (np.eye(S, dtype=f) - mm) @ i['ls1w'].T.astype(f)

    Msh = np.zeros((128, MCOLS), f)
    Mbf = np.zeros((128, MCOLS), ml_dtypes.bfloat16)

    # wa u-major blocks
    for u in range(3):
        for j in range(3):
            Msh[0:KC, WA0 + S * u + KC * j:WA0 + S * u + KC * (j + 1)] = \
                w1[KC * j:KC * (j + 1), KC * u:KC * (u + 1)]

    # constant detail_pred row (LayerNorm(1) output == ln_b exactly)
    xf = np.full((S,), f(i['ln_b'][0]), f)
    dp_row = (np.maximum(xf @ i['op1w'].T + i['op1b'], 0)
              @ i['op2w'].T + i['op2b']).astype(f)
    dpm = dp_row.mean(dtype=np.float32)
    b1f = (i['fn1b'] + dpm * i['fn1w'][:, 2 * C:].sum(1)).astype(f)

    lt2wt = np.ascontiguousarray(i['lt2w'].T, f)
    ls2wt = np.ascontiguousarray(i['ls2w'].T, f)
    # w2full [337, 194] = [tp 0:96 | sp 96:192 | tps 192 | sps 193]
    w2full = np.zeros((S + 1, 194), f)
    w2full[0:HID, 0:96] = lt2wt
    w2full[0:HID, 192] = lt2wt.sum(1)
    w2full[HID:S, 96:192] = ls2wt
    w2full[HID:S, 193] = ls2wt.sum(1)
    w2full[S, 0:96] = i['lt2b']
    w2full[S, 192] = i['lt2b'].sum(dtype=np.float64)
    w2full[S, 96:192] = i['ls2b']
    w2full[S, 193] = i['ls2b'].sum(dtype=np.float64)
    for u in range(3):
        Msh[0:KC, WIDE0 + 192 * u:WIDE0 + 192 * (u + 1)] = \
            w2full[KC * u:KC * (u + 1), 0:192]
        Msh[0:KC, SUM0 + 2 * u:SUM0 + 2 * (u + 1)] = \
            w2full[KC * u:KC * (u + 1), 192:194]
    Msh[KC, WIDE0 + 384:WIDE0 + 576] = w2full[S, 0:192]
    Msh[KC, SUM0 + 4:SUM0 + 6] = w2full[S, 192:194]

    # fn1 aug (bias b1f rides row 96 of fn1t; ts2 row 96 == 1)
    Msh[0:96, FN1T:FN1T + 32] = i['fn1w'][:, 0:C].T / C
    Msh[96, FN1T:FN1T + 32] = b1f
    Msh[0:96, FN1S:FN1S + 32] = i['fn1w'][:, C:2 * C].T / C

    fn2T = np.ascontiguousarray(i['fn2w'].T, f)
    Msh[0:32, FN2:FN2 + 288] = fn2T
    Msh[32, FN2:FN2 + 288] = i['fn2b']

    Msh[0:96, FP1W:FP1W + 48] = i['fp1w'].T
    Msh[0:96, DPB:DPB + 96] = np.broadcast_to(dp_row[None, :], (96, 96))
    Msh[0:48, FP2:FP2 + 96] = i['fp2w'].T
    Msh[48, FP2:FP2 + 96] = i['fp2b']

    Mbf[:, :] = Msh.astype(ml_dtypes.bfloat16)

    # f32-bit-packed columns (overwrite the bf16 rounding)
    b1 = np.concatenate([i['lt1b'], i['ls1b']]).astype(f)
    for u in range(3):
        Mbf[0:KC, B1B + 2 * u:B1B + 2 * (u + 1)] = \
            _f32bits(b1[KC * u:KC * (u + 1)][:, None])
    Mbf[0:48, FP1B:FP1B + 2] = _f32bits(i['fp1b'].astype(f)[:, None])

    return Mbf


def _chunks(Mbf):
    cc = np.ascontiguousarray
    return dict(
        T2=cc(Mbf[0:112, WA0:WA0 + S]),
        T3=cc(Mbf[0:112, WA0 + S:WA0 + 2 * S]),
        T4=cc(Mbf[0:112, WA0 + 2 * S:WA0 + 3 * S]),
        T5=cc(Mbf[0:128, SUM0:WIDE0]),
        T6a=cc(Mbf[0:112, WIDE0:WIDE0 + 192]),
        T6b=cc(Mbf[0:128, WIDE0 + 192:FN2]),
        T8=cc(Mbf[0:48, FN2:FP1W]),
        T9=cc(Mbf[0:96, FP1W:MCOLS]),
    )


def make_in_maps(inputs):
    Mbf = _prep_weights(inputs)
    shared = _chunks(Mbf)
    x = np.asarray(inputs['x'], np.float32)
    in_maps = []
    for b in range(N_CORES):
        T1 = np.ascontiguousarray(Mbf[0:112, 0:WA0])
        for j in range(3):
            T1[0:KC, C * j:C * (j + 1)] = _bf(x[b, KC * j:KC * (j + 1), :])
        in_maps.append(dict(shared, T1=T1))
    return in_maps


def kernel(**inputs):
    if "nc" not in _CACHE:
        _CACHE["nc"] = _build_module()
    res = run_bass_kernel_spmd(_CACHE["nc"], make_in_maps(inputs),
                               core_ids=list(range(N_CORES)))
    out = np.empty((N_CORES, P, P), np.float32)
    for b in range(N_CORES):
        y2 = res.results[b]["y"]
        out[b, 0:48, :] = y2[:, 0:96]
        out[b, 48:96, :] = y2[:, 96:192]
    return out
